# revision 11
# baseline (speedup 1.0000x reference)
"""Trainium2 Bass kernel: segment-reduced Euclidean loss.

loss = sum_i ||a_i - b_i||_2 / num_list[seg(i)]   over N rows, D=128.

Strategy (8 NeuronCores, data-parallel, fp8 streams):
  - rows split evenly across 8 cores (segment boundaries preserved).
  - streams are cast to fp8-e4m3 on the host AND transposed per core to
    [128=d, rows] so the embedding dim sits on SBUF partitions. End-to-end
    fp8 rel-err measured 3.8e-4 (gate 2e-2). HBM floor: 2 x 16 MiB per
    core / 358 GB/s = ~94 us (vs ~187 us for the fp16 baseline).
  - per 8192-col chunk (cols = rows in transposed layout):
      DMA a,b fp8 (~6.3 us) ->
      diff = a-b in fp16: split DVE (cols [0,SD), 1x rate on fp8 input)
        and GPSIMD (cols [SD,cu)) so neither exceeds the DMA budget ->
      square in place: DVE tensor_mul on [0,SM) (2x fp16), ACT Square on
        [SM,cu) ->
      PE: per 128 cols one matmul with the squared-diff block as the
        STATIONARY operand and a ones[128,1] moving vector: out [128,1]
        = per-row sum over d, partition-spread, written to one column of
        a persistent PSUM [128,1024] f32 (2 banks). Row r lands at PSUM
        (partition r%128, column r//128). LS streams the data through
        the PE array at ~1 col/cycle, so PE cost ~= the ones-matmul.
  - tail: ACT Sqrt on PSUM [128,1024] -> SBUF, multiply by per-row
    weight 1/num_list[seg(row)] (host sends w[m,t] = wrow[128t+m]),
    row-reduce, DMA [128,1] out. Host sums 8x128 partials in f64.
  - taper: last chunk split into 4x2048 cols so the pipeline drain after
    the final DMA is short.
"""

import numpy as np

N_ROWS = 1048576
D = 128
N_SEG = 2048
N_CORES = 8
ROWS_PER_CORE = N_ROWS // N_CORES  # 131072
CHUNK = 8192          # columns (rows of the original tensor) per chunk
MM_N = 512            # moving free size per matmul (PSUM bank)
SD_FRAC = 0.625       # fraction of chunk cols subtracted on DVE (rest GPSIMD)
SM_FRAC = 0.1875      # fraction of chunk cols squared on DVE (rest ACT)


def _split_excess_waits(nc, max_waits=1):
    """walrus in this container rejects instructions carrying more than 1
    sync-wait condition. Move excess waits onto NoOp carriers just before
    the offender on the same engine (same-engine program order keeps it
    semantically identical)."""
    import concourse.mybir as mybir

    for f in nc.m.functions:
        for bb in f.blocks:
            out = []
            changed = False
            for inst in bb.instructions:
                si = inst.sync_info
                waits = list(si.on_wait) if si is not None else []
                if len(waits) > max_waits:
                    keep = waits[-max_waits:]
                    extra = waits[:-max_waits]
                    k = 0
                    while extra:
                        take, extra = extra[:max_waits], extra[max_waits:]
                        nop = mybir.InstNoOp(name=f"{inst.name}-wsplit{k}")
                        nop.engine = inst.engine
                        nop.sync_info = mybir.SyncInfo(on_wait=take, on_update=[])
                        out.append(nop)
                        k += 1
                    inst.sync_info = mybir.SyncInfo(
                        on_wait=keep, on_update=list(si.on_update)
                    )
                    changed = True
                out.append(inst)
            if changed:
                bb.instructions = out


def _chunk_sizes(total_cols, chunk=CHUNK):
    n_chunk = total_cols // chunk
    assert n_chunk * chunk == total_cols
    sizes = [chunk] * n_chunk
    # taper the tail so the post-last-DMA compute chain is short
    if n_chunk >= 2 and chunk % (4 * MM_N) == 0:
        sizes = [chunk] * (n_chunk - 1) + [chunk // 4] * 4
    return sizes


def build_nc(rows_per_core=ROWS_PER_CORE, chunk=CHUNK, bufs=4, iters=1,
             sd_frac=SD_FRAC, sm_frac=SM_FRAC, taper=True):
    """Per-core SPMD Bass program (same program on every core).

    iters>1 repeats the streaming loop (same data) for slope timing."""
    import concourse.bass as bass
    import concourse.mybir as mybir
    import concourse.tile as tile

    R = rows_per_core
    q = R // 128              # norms^2 per partition / PSUM cols (1024)
    assert q * 128 == R
    f32 = mybir.dt.float32
    f16 = mybir.dt.float16
    f8 = mybir.dt.float8e4
    AF = mybir.ActivationFunctionType

    sizes = _chunk_sizes(R, chunk) if taper else [chunk] * (R // chunk)

    nc = bass.Bass("TRN2", target_bir_lowering=False, debug=False)
    a = nc.declare_dram_parameter("a", [128, R], f8, isOutput=False)
    b = nc.declare_dram_parameter("b", [128, R], f8, isOutput=False)
    w = nc.declare_dram_parameter("w", [128, q], f32, isOutput=False)
    o = nc.declare_dram_parameter("o", [128, 1], f32, isOutput=True)

    with tile.TileContext(nc) as tc:
        with (
            tc.tile_pool(name="pa", bufs=bufs) as pa,
            tc.tile_pool(name="pb", bufs=bufs) as pb,
            tc.tile_pool(name="pd", bufs=2) as pd,
            tc.tile_pool(name="pp", bufs=1) as pp,
            tc.psum_pool(name="pn", bufs=1) as pn,
        ):
            ones = pp.tile([128, 1], f16, tag="ones")
            wt = pp.tile([128, q], f32, tag="wt")
            sn = pp.tile([128, q], f32, tag="sn")
            acc = pp.tile([128, 1], f32, tag="acc")
            norms2 = pn.tile([128, q], f32, tag="norms2")

            nc.vector.memset(ones[:], 1.0)
            # weights ride the ACT HWDGE ring so the SP ring head stays free
            nc.scalar.dma_start(out=wt[:], in_=w[:])

            for _ in range(iters):
                t_blk = 0  # global 128-col block counter -> PSUM column
                off = 0
                for cu in sizes:
                    ta = pa.tile([128, chunk], f8)
                    tb = pb.tile([128, chunk], f8)
                    td = pd.tile([128, chunk], f16)
                    nc.sync.dma_start(out=ta[:, :cu], in_=a[:, off:off + cu])
                    nc.sync.dma_start(out=tb[:, :cu], in_=b[:, off:off + cu])
                    sd = (int(cu * sd_frac) // 128) * 128
                    sm = (int(cu * sm_frac) // 128) * 128
                    # diff = a - b (fp16 out). DVE head, GPSIMD tail.
                    nc.vector.tensor_sub(td[:, :sd], ta[:, :sd], tb[:, :sd])
                    if sd < cu:
                        nc.gpsimd.tensor_sub(td[:, sd:cu], ta[:, sd:cu],
                                             tb[:, sd:cu])
                    # square in place. DVE head (2x fp16), ACT tail.
                    if sm > 0:
                        nc.vector.tensor_mul(td[:, :sm], td[:, :sm], td[:, :sm])
                    nc.scalar.activation(td[:, sm:cu], td[:, sm:cu], AF.Square)
                    # PE: data-stationary reduce: 128-col block as lhsT, ones
                    # moving -> out [128,1] = per-row norms^2 at PSUM col t.
                    for j in range(cu // 128):
                        nc.tensor.matmul(
                            norms2[:, t_blk % q:t_blk % q + 1],
                            td[:, j * 128:(j + 1) * 128],
                            ones[:],
                            start=True, stop=True,
                        )
                        t_blk += 1
                    off += cu

            nc.scalar.activation(sn[:], norms2[:], AF.Sqrt)
            nc.vector.tensor_mul(sn[:], sn[:], wt[:])
            nc.vector.tensor_reduce(
                acc[:], sn[:], axis=mybir.AxisListType.X,
                op=mybir.AluOpType.add
            )
            nc.sync.dma_start(out=o[:], in_=acc[:])

    _split_excess_waits(nc)
    return nc


_CACHE = {}


def _get_nc(rows_per_core, chunk=CHUNK):
    key = (rows_per_core, chunk)
    if key not in _CACHE:
        _CACHE[key] = build_nc(rows_per_core, chunk)
    return _CACHE[key]


def _seg_ids(num_list, n_rows):
    """np.repeat with jnp.repeat(total_repeat_length=n) pad/truncate."""
    nl = np.asarray(num_list, dtype=np.int64)
    full = np.repeat(np.arange(nl.shape[0], dtype=np.int64), nl)
    if full.size >= n_rows:
        return full[:n_rows]
    pad_val = full[-1] if full.size else 0
    return np.concatenate([full, np.full(n_rows - full.size, pad_val, np.int64)])


def _psum_perm(rows_per_core):
    """idx[m, t] = row index whose norm^2 lands at PSUM (m, t) = 128t+m."""
    q = rows_per_core // 128
    return (np.arange(128)[:, None] + 128 * np.arange(q)[None, :]).astype(np.int64)


def make_in_maps(clip_remap, clip_emb, num_list, rows_per_core=ROWS_PER_CORE):
    import concourse.mybir as mybir

    f8np = mybir.dt.np(mybir.dt.float8e4)
    a8 = np.asarray(clip_remap, dtype=np.float32).astype(f8np)
    b8 = np.asarray(clip_emb, dtype=np.float32).astype(f8np)
    n_rows = a8.shape[0]
    nl = np.asarray(num_list)
    seg = _seg_ids(nl, n_rows)
    denom = nl[seg].astype(np.float32)
    wrow = (np.float32(1.0) / denom).astype(np.float32)
    idx = _psum_perm(rows_per_core)
    in_maps = []
    for c in range(N_CORES):
        lo, hi = c * rows_per_core, (c + 1) * rows_per_core
        in_maps.append(
            {
                "a": np.ascontiguousarray(a8[lo:hi].T),
                "b": np.ascontiguousarray(b8[lo:hi].T),
                "w": np.ascontiguousarray(wrow[lo:hi][idx]),
            }
        )
    return in_maps


_RUNNER_CACHE = {}


def _get_runner(rows_per_core, chunk=CHUNK):
    """Compile once per process; reuse the jitted SPMD executable across
    kernel() calls."""
    key = (rows_per_core, chunk)
    if key in _RUNNER_CACHE:
        return _RUNNER_CACHE[key]

    import jax
    from jax.experimental.shard_map import shard_map
    from jax.sharding import Mesh, NamedSharding, PartitionSpec

    import concourse.bass2jax as b2j
    import concourse.mybir as mybir

    b2j.install_neuronx_cc_hook()
    nc = _get_nc(rows_per_core, chunk)

    in_names, out_names, out_avals, zero_outs = [], [], [], []
    pname = nc.partition_id_tensor.name if nc.partition_id_tensor else None
    for alloc in nc.m.functions[0].allocations:
        if not isinstance(alloc, mybir.MemoryLocationSet):
            continue
        name = alloc.memorylocations[0].name
        if alloc.kind == "ExternalInput":
            if name != pname:
                in_names.append(name)
        elif alloc.kind == "ExternalOutput":
            out_names.append(name)
            shape = tuple(alloc.tensor_shape)
            dtype = mybir.dt.np(alloc.dtype)
            out_avals.append(jax.core.ShapedArray(shape, dtype))
            zero_outs.append(np.zeros(shape, dtype))
    n_params = len(in_names)
    all_in = list(in_names) + list(out_names)
    if pname is not None:
        all_in.append(pname)

    def _body(*args):
        operands = list(args)
        if pname is not None:
            operands.append(b2j.partition_id_tensor())
        return tuple(
            b2j._bass_exec_p.bind(
                *operands,
                out_avals=tuple(out_avals),
                in_names=tuple(all_in),
                out_names=tuple(out_names),
                lowering_input_output_aliases=(),
                sim_require_finite=True,
                sim_require_nnan=True,
                nc=nc,
            )
        )

    devices = jax.devices()[:N_CORES]
    mesh = Mesh(np.asarray(devices), ("core",))
    n_outs = len(out_avals)
    fn = jax.jit(
        shard_map(
            _body,
            mesh=mesh,
            in_specs=(PartitionSpec("core"),) * (n_params + n_outs),
            out_specs=(PartitionSpec("core"),) * n_outs,
            check_rep=False,
        ),
        keep_unused=True,
    )
    sh = NamedSharding(mesh, PartitionSpec("core"))

    def run(in_maps):
        dev_in = [
            jax.device_put(
                np.concatenate([np.asarray(m[nm]) for m in in_maps], axis=0), sh
            )
            for nm in in_names
        ]
        dev_zero = [
            jax.device_put(np.concatenate([z] * N_CORES, axis=0), sh)
            for z in zero_outs
        ]
        outs = fn(*dev_in, *dev_zero)
        jax.block_until_ready(outs)
        results = []
        for c in range(N_CORES):
            r = {}
            for i, nm in enumerate(out_names):
                arr = np.asarray(outs[i])
                per = arr.shape[0] // N_CORES
                r[nm] = arr[c * per:(c + 1) * per]
            results.append(r)
        return results

    _RUNNER_CACHE[key] = run
    return run


def kernel(clip_remap, clip_emb, num_list):
    a = np.asarray(clip_remap)
    rows_per_core = a.shape[0] // N_CORES
    in_maps = make_in_maps(clip_remap, clip_emb, num_list, rows_per_core)
    results = None
    last_err = None
    for attempt in range(4):
        try:
            if attempt < 3:
                run = _get_runner(rows_per_core, CHUNK)
                results = run(in_maps)
            else:
                from concourse.bass_utils import run_bass_kernel_spmd

                res = run_bass_kernel_spmd(
                    _get_nc(rows_per_core, CHUNK),
                    in_maps,
                    core_ids=list(range(N_CORES)),
                )
                results = res.results
            break
        except Exception as e:  # transient NRT/axon failures observed
            last_err = e
            import time as _time

            _time.sleep(2.0 * (attempt + 1))
            if attempt >= 1:
                _RUNNER_CACHE.pop((rows_per_core, CHUNK), None)
    if results is None:
        raise last_err
    total = np.float64(0.0)
    for r in results:
        total += r["o"].astype(np.float64).sum()
    return np.asarray(total, dtype=np.float32)


# revision 24
# speedup vs baseline: 1.2597x; 1.2597x over previous
"""Trainium2 Bass kernel: segment-reduced Euclidean loss.

loss = sum_i ||a_i - b_i||_2 / num_list[seg(i)]   over N rows, D=128.

Strategy (8 NeuronCores, data-parallel, fp8 streams):
  - rows split evenly across 8 cores (segment boundaries preserved).
  - streams are cast to fp8-e4m3 on the host AND transposed per core to
    [128=d, rows] so the embedding dim sits on SBUF partitions. End-to-end
    fp8 rel-err measured 3.8e-4 (gate 2e-2). HBM floor: 2 x 16 MiB per
    core / 358 GB/s = ~94 us (vs ~187 us for the fp16 baseline).
  - per 8192-col chunk (cols = rows in transposed layout):
      DMA a,b fp8 (~6.3 us) ->
      diff = a-b in fp16: split DVE (cols [0,SD), 1x rate on fp8 input)
        and GPSIMD (cols [SD,cu)) so neither exceeds the DMA budget ->
      square in place: DVE tensor_mul on [0,SM) (2x fp16), ACT Square on
        [SM,cu) ->
      PE: per 128 cols one matmul with the squared-diff block as the
        STATIONARY operand and a ones[128,1] moving vector: out [128,1]
        = per-row sum over d, partition-spread, written to one column of
        a persistent PSUM [128,1024] f32 (2 banks). Row r lands at PSUM
        (partition r%128, column r//128). LS streams the data through
        the PE array at ~1 col/cycle, so PE cost ~= the ones-matmul.
  - tail: ACT Sqrt on PSUM [128,1024] -> SBUF, multiply by per-row
    weight 1/num_list[seg(row)] (host sends w[m,t] = wrow[128t+m]),
    row-reduce, DMA [128,1] out. Host sums 8x128 partials in f64.
  - taper: last chunk split into 4x2048 cols so the pipeline drain after
    the final DMA is short.
"""

import numpy as np

N_ROWS = 1048576
D = 128
N_SEG = 2048
N_CORES = 8
ROWS_PER_CORE = N_ROWS // N_CORES  # 131072
CHUNK = 8192          # columns (rows of the original tensor) per chunk
MM_N = 512            # moving free size per matmul (PSUM bank)
SD_FRAC = 1.0         # fraction of chunk cols subtracted on DVE (rest GPSIMD;
                      # 1.0 = no GPSIMD -- its SBUF port contention with DVE
                      # serialized the pipeline on HW: 187us vs 98.5us without)
SM_FRAC = 0.21875     # fraction of chunk cols squared on DVE (rest ACT)
PACK = 1              # fp8 elements per DMA container element (1, 2, or 4)


def _split_excess_waits(nc, max_waits=1):
    """walrus in this container rejects instructions carrying more than 1
    sync-wait condition. Move excess waits onto NoOp carriers just before
    the offender on the same engine (same-engine program order keeps it
    semantically identical)."""
    import concourse.mybir as mybir

    for f in nc.m.functions:
        for bb in f.blocks:
            out = []
            changed = False
            for inst in bb.instructions:
                si = inst.sync_info
                waits = list(si.on_wait) if si is not None else []
                if len(waits) > max_waits:
                    keep = waits[-max_waits:]
                    extra = waits[:-max_waits]
                    k = 0
                    while extra:
                        take, extra = extra[:max_waits], extra[max_waits:]
                        nop = mybir.InstNoOp(name=f"{inst.name}-wsplit{k}")
                        nop.engine = inst.engine
                        nop.sync_info = mybir.SyncInfo(on_wait=take, on_update=[])
                        out.append(nop)
                        k += 1
                    inst.sync_info = mybir.SyncInfo(
                        on_wait=keep, on_update=list(si.on_update)
                    )
                    changed = True
                out.append(inst)
            if changed:
                bb.instructions = out


def _chunk_sizes(total_cols, chunk=CHUNK):
    n_chunk = total_cols // chunk
    assert n_chunk * chunk == total_cols
    sizes = [chunk] * n_chunk
    # taper the tail so the post-last-DMA compute chain is short
    if n_chunk >= 2 and chunk % (4 * MM_N) == 0:
        sizes = [chunk] * (n_chunk - 1) + [chunk // 4] * 4
    return sizes


def build_nc(rows_per_core=ROWS_PER_CORE, chunk=CHUNK, bufs=4, iters=1,
             sd_frac=SD_FRAC, sm_frac=SM_FRAC, taper=True, pack=PACK,
             no_pe=False, ring2=None):
    """Per-core SPMD Bass program (same program on every core).

    iters>1 repeats the streaming loop (same data) for slope timing."""
    import concourse.bass as bass
    import concourse.mybir as mybir
    import concourse.tile as tile

    R = rows_per_core
    q = R // 128              # norms^2 per partition / PSUM cols (1024)
    assert q * 128 == R
    f32 = mybir.dt.float32
    f16 = mybir.dt.float16
    f8 = mybir.dt.float8e4
    AF = mybir.ActivationFunctionType

    sizes = _chunk_sizes(R, chunk) if taper else [chunk] * (R // chunk)

    pdt = {1: f8, 2: f16, 4: f32}[pack]  # DMA container dtype

    nc = bass.Bass("TRN2", target_bir_lowering=False, debug=False)
    a = nc.declare_dram_parameter("a", [128, R // pack], pdt, isOutput=False)
    b = nc.declare_dram_parameter("b", [128, R // pack], pdt, isOutput=False)
    w = nc.declare_dram_parameter("w", [128, q], f32, isOutput=False)
    o = nc.declare_dram_parameter("o", [128, 1], f32, isOutput=True)

    with tile.TileContext(nc) as tc:
        with (
            tc.tile_pool(name="pa", bufs=bufs) as pa,
            tc.tile_pool(name="pb", bufs=bufs) as pb,
            tc.tile_pool(name="pd", bufs=2) as pd,
            tc.tile_pool(name="pp", bufs=1) as pp,
            tc.psum_pool(name="pn", bufs=1) as pn,
        ):
            ones = pp.tile([128, 1], f16, tag="ones")
            wt = pp.tile([128, q], f32, tag="wt")
            sn = pp.tile([128, q], f32, tag="sn")
            acc = pp.tile([128, 1], f32, tag="acc")
            if no_pe:
                norms2 = None
            else:
                norms2 = pn.tile([128, q], f32, tag="norms2")

            nc.vector.memset(ones[:], 1.0)
            # weights ride the ACT HWDGE ring so the SP ring head stays free
            nc.scalar.dma_start(out=wt[:], in_=w[:])

            for _ in range(iters):
                t_blk = 0  # global 128-col block counter -> PSUM column
                off = 0
                for cu in sizes:
                    tap = pa.tile([128, chunk // pack], pdt)
                    tbp = pb.tile([128, chunk // pack], pdt)
                    td = pd.tile([128, chunk], f16)
                    eng2 = getattr(nc, ring2) if ring2 else nc.sync
                    nc.sync.dma_start(out=tap[:, :cu // pack],
                                      in_=a[:, off // pack:(off + cu) // pack])
                    eng2.dma_start(out=tbp[:, :cu // pack],
                                   in_=b[:, off // pack:(off + cu) // pack])
                    ta = tap[:].bitcast(f8)
                    tb = tbp[:].bitcast(f8)
                    sd = (int(cu * sd_frac) // 128) * 128
                    sm = (int(cu * sm_frac) // 128) * 128
                    # diff = a - b (fp16 out). DVE head, GPSIMD tail.
                    nc.vector.tensor_sub(td[:, :sd], ta[:, :sd], tb[:, :sd])
                    if sd < cu:
                        nc.gpsimd.tensor_sub(td[:, sd:cu], ta[:, sd:cu],
                                             tb[:, sd:cu])
                    # square in place. DVE head (2x fp16), ACT tail.
                    if sm > 0:
                        nc.vector.tensor_mul(td[:, :sm], td[:, :sm], td[:, :sm])
                    nc.scalar.activation(td[:, sm:cu], td[:, sm:cu], AF.Square)
                    # PE: data-stationary reduce: 128-col block as lhsT, ones
                    # moving -> out [128,1] = per-row norms^2 at PSUM col t.
                    if no_pe:
                        # timing ablation: consume td cheaply instead
                        nc.vector.tensor_copy(acc[:], td[:, :1])
                        t_blk += cu // 128
                    else:
                        for j in range(cu // 128):
                            nc.tensor.matmul(
                                norms2[:, t_blk % q:t_blk % q + 1],
                                td[:, j * 128:(j + 1) * 128],
                                ones[:],
                                start=True, stop=True,
                            )
                            t_blk += 1
                    off += cu

            src = wt if no_pe else norms2
            nc.scalar.activation(sn[:], src[:], AF.Sqrt)
            nc.vector.tensor_mul(sn[:], sn[:], wt[:])
            nc.vector.tensor_reduce(
                acc[:], sn[:], axis=mybir.AxisListType.X,
                op=mybir.AluOpType.add
            )
            nc.sync.dma_start(out=o[:], in_=acc[:])

    _split_excess_waits(nc)
    return nc


_CACHE = {}


def _get_nc(rows_per_core, chunk=CHUNK):
    key = (rows_per_core, chunk)
    if key not in _CACHE:
        _CACHE[key] = build_nc(rows_per_core, chunk)
    return _CACHE[key]


def _seg_ids(num_list, n_rows):
    """np.repeat with jnp.repeat(total_repeat_length=n) pad/truncate."""
    nl = np.asarray(num_list, dtype=np.int64)
    full = np.repeat(np.arange(nl.shape[0], dtype=np.int64), nl)
    if full.size >= n_rows:
        return full[:n_rows]
    pad_val = full[-1] if full.size else 0
    return np.concatenate([full, np.full(n_rows - full.size, pad_val, np.int64)])


def _psum_perm(rows_per_core):
    """idx[m, t] = row index whose norm^2 lands at PSUM (m, t) = 128t+m."""
    q = rows_per_core // 128
    return (np.arange(128)[:, None] + 128 * np.arange(q)[None, :]).astype(np.int64)


def make_in_maps(clip_remap, clip_emb, num_list, rows_per_core=ROWS_PER_CORE):
    import concourse.mybir as mybir

    f8np = mybir.dt.np(mybir.dt.float8e4)
    pnp = {1: None, 2: np.float16, 4: np.float32}[PACK]
    a8 = np.asarray(clip_remap, dtype=np.float32).astype(f8np)
    b8 = np.asarray(clip_emb, dtype=np.float32).astype(f8np)
    n_rows = a8.shape[0]
    nl = np.asarray(num_list)
    seg = _seg_ids(nl, n_rows)
    denom = nl[seg].astype(np.float32)
    wrow = (np.float32(1.0) / denom).astype(np.float32)
    idx = _psum_perm(rows_per_core)
    in_maps = []
    for c in range(N_CORES):
        lo, hi = c * rows_per_core, (c + 1) * rows_per_core
        at = np.ascontiguousarray(a8[lo:hi].T)
        bt = np.ascontiguousarray(b8[lo:hi].T)
        if pnp is not None:
            at = at.view(np.uint8).view(pnp)
            bt = bt.view(np.uint8).view(pnp)
        in_maps.append(
            {
                "a": at,
                "b": bt,
                "w": np.ascontiguousarray(wrow[lo:hi][idx]),
            }
        )
    return in_maps


_RUNNER_CACHE = {}


def _get_runner(rows_per_core, chunk=CHUNK):
    """Compile once per process; reuse the jitted SPMD executable across
    kernel() calls."""
    key = (rows_per_core, chunk)
    if key in _RUNNER_CACHE:
        return _RUNNER_CACHE[key]

    import jax
    from jax.experimental.shard_map import shard_map
    from jax.sharding import Mesh, NamedSharding, PartitionSpec

    import concourse.bass2jax as b2j
    import concourse.mybir as mybir

    b2j.install_neuronx_cc_hook()
    nc = _get_nc(rows_per_core, chunk)

    in_names, out_names, out_avals, zero_outs = [], [], [], []
    pname = nc.partition_id_tensor.name if nc.partition_id_tensor else None
    for alloc in nc.m.functions[0].allocations:
        if not isinstance(alloc, mybir.MemoryLocationSet):
            continue
        name = alloc.memorylocations[0].name
        if alloc.kind == "ExternalInput":
            if name != pname:
                in_names.append(name)
        elif alloc.kind == "ExternalOutput":
            out_names.append(name)
            shape = tuple(alloc.tensor_shape)
            dtype = mybir.dt.np(alloc.dtype)
            out_avals.append(jax.core.ShapedArray(shape, dtype))
            zero_outs.append(np.zeros(shape, dtype))
    n_params = len(in_names)
    all_in = list(in_names) + list(out_names)
    if pname is not None:
        all_in.append(pname)

    def _body(*args):
        operands = list(args)
        if pname is not None:
            operands.append(b2j.partition_id_tensor())
        return tuple(
            b2j._bass_exec_p.bind(
                *operands,
                out_avals=tuple(out_avals),
                in_names=tuple(all_in),
                out_names=tuple(out_names),
                lowering_input_output_aliases=(),
                sim_require_finite=True,
                sim_require_nnan=True,
                nc=nc,
            )
        )

    devices = jax.devices()[:N_CORES]
    mesh = Mesh(np.asarray(devices), ("core",))
    n_outs = len(out_avals)
    fn = jax.jit(
        shard_map(
            _body,
            mesh=mesh,
            in_specs=(PartitionSpec("core"),) * (n_params + n_outs),
            out_specs=(PartitionSpec("core"),) * n_outs,
            check_rep=False,
        ),
        keep_unused=True,
    )
    sh = NamedSharding(mesh, PartitionSpec("core"))

    def run(in_maps):
        dev_in = [
            jax.device_put(
                np.concatenate([np.asarray(m[nm]) for m in in_maps], axis=0), sh
            )
            for nm in in_names
        ]
        dev_zero = [
            jax.device_put(np.concatenate([z] * N_CORES, axis=0), sh)
            for z in zero_outs
        ]
        outs = fn(*dev_in, *dev_zero)
        jax.block_until_ready(outs)
        results = []
        for c in range(N_CORES):
            r = {}
            for i, nm in enumerate(out_names):
                arr = np.asarray(outs[i])
                per = arr.shape[0] // N_CORES
                r[nm] = arr[c * per:(c + 1) * per]
            results.append(r)
        return results

    _RUNNER_CACHE[key] = run
    return run


def kernel(clip_remap, clip_emb, num_list):
    a = np.asarray(clip_remap)
    rows_per_core = a.shape[0] // N_CORES
    in_maps = make_in_maps(clip_remap, clip_emb, num_list, rows_per_core)
    results = None
    last_err = None
    for attempt in range(4):
        try:
            if attempt < 3:
                run = _get_runner(rows_per_core, CHUNK)
                results = run(in_maps)
            else:
                from concourse.bass_utils import run_bass_kernel_spmd

                res = run_bass_kernel_spmd(
                    _get_nc(rows_per_core, CHUNK),
                    in_maps,
                    core_ids=list(range(N_CORES)),
                )
                results = res.results
            break
        except Exception as e:  # transient NRT/axon failures observed
            last_err = e
            import time as _time

            _time.sleep(2.0 * (attempt + 1))
            if attempt >= 1:
                _RUNNER_CACHE.pop((rows_per_core, CHUNK), None)
    if results is None:
        raise last_err
    total = np.float64(0.0)
    for r in results:
        total += r["o"].astype(np.float64).sum()
    return np.asarray(total, dtype=np.float32)
